# revision 35
# baseline (speedup 1.0000x reference)
"""GATv2 x2 + global mean pool on 8 Trainium2 NeuronCores (Bass/Tile), v2.

Slot-space layout (dst-sharded, uniform tiles):
  - Edges sorted by dst; per core, greedy tiles of <=8 dst nodes ("window
    slots") and <=128 edges. Tile t owns slot rows [8t, 8t+8); a batch of
    16 tiles = 128 consecutive slot rows, so ALL window-side accesses
    (xr rows, pool indicators) are plain contiguous DMAs.
  - Node tables (xl/xr) are computed in slot space; xl tables are
    AllGathered to a [8*S, 256] global table; per-edge xl rows come via
    per-tile indirect DMA gathers with global-slot indices.
  - Scores: tables have no aux cols; e = sum_hd att*leaky(S) computed as
    Prelu(S) on ACT, att-mul + binary-tree head reduction on DVE.
  - Aggregation: alpha is folded into the one-hot indicator (i0a = i0 *
    alpha per head), so num = sum per head via 4 matmuls per tile and the
    per-edge [128,256] alpha-broadcast multiply disappears.
  - silu via exp table (1/(1+e^-x)*x) to keep a single ACT table set
    (prelu/exp/copy) with zero table reloads.
  - Layer-2 transforms are fused into the layer-1 edge loop: h tiles are
    PE-transposed in SBUF and immediately transformed; h never goes to
    DRAM.
  - Global mean pool via per-batch [128,64] indicator matmul accumulated
    in PSUM across all batches; final AllReduce over [64,256].
"""
import sys

sys.path.insert(0, "/opt/trn_rl_repo")
sys.path.insert(0, "/opt/pypackages")

import os
from contextlib import ExitStack

import numpy as np
import ml_dtypes

import concourse.bass as bass
import concourse.mybir as mybir
import concourse.tile as tile

BF16 = ml_dtypes.bfloat16
bf = mybir.dt.bfloat16
f32 = mybir.dt.float32
i32 = mybir.dt.int32

N, E, G = 50000, 800000, 64
IN, H, D = 128, 4, 64
HD = H * D
NCORES = 8
NPC = N // NCORES
P = 128                      # edge slots per tile
W = 8                        # window (dst-node) slots per tile
GB = 16                      # tiles per batch (= 128 slot rows)
NEG = 0.2

# ---------------------------------------------------------------- host prep


def _tile_core(dst_l):
    """Bin-pack local nodes into tiles of <=W nodes and <=P edges.

    Nodes are relabeled freely within the core (everything downstream is
    slot-indexed). Snake-deal by degree, then repair overloaded bins.
    Returns (tiles, counts): tiles = list of node-id arrays.
    """
    counts = np.bincount(dst_l, minlength=NPC)
    assert counts.max() <= P, counts.max()
    order = np.argsort(-counts, kind="stable")
    import heapq
    nb = max((NPC + W - 1) // W, int(np.ceil(counts.sum() / P)))
    while True:
        bins = [[] for _ in range(nb)]
        load = np.zeros(nb, np.int64)
        heap = [(0, b) for b in range(nb)]
        heapq.heapify(heap)
        ok = True
        for n in order:
            c = int(counts[n])
            parked = []
            placed = False
            while heap:
                ld, b = heapq.heappop(heap)
                if ld + c <= P and len(bins[b]) < W:
                    bins[b].append(n)
                    load[b] = ld + c
                    if len(bins[b]) < W:
                        heapq.heappush(heap, (ld + c, b))
                    placed = True
                    break
                parked.append((ld, b))
                if ld + c > P:
                    break
            for it in parked:
                heapq.heappush(heap, it)
            if not placed:
                ok = False
                break
        if ok:
            return [np.sort(np.asarray(b, np.int64)) for b in bins], counts
        nb += 4


def _host_prep(inputs):
    x = np.asarray(inputs["x"], np.float32)
    ei = np.asarray(inputs["edge_index"]).astype(np.int64)
    batch = np.asarray(inputs["batch"]).astype(np.int64)

    src, dst = ei[0], ei[1]
    order = np.argsort(dst, kind="stable")
    src_s, dst_s = src[order].astype(np.int64), dst[order].astype(np.int64)
    core_of = dst_s // NPC

    cnt = np.bincount(batch, minlength=G).astype(np.float32)
    inv_cnt = (1.0 / np.maximum(cnt, 1.0)).astype(np.float32)

    per_core = []
    tiles_pc = []
    for k in range(NCORES):
        m = core_of == k
        s_k, d_k = src_s[m], dst_s[m] - k * NPC
        tiles, counts = _tile_core(d_k)
        per_core.append((s_k, d_k, counts))
        tiles_pc.append(tiles)

    T = max(len(t) for t in tiles_pc)
    T = ((T + GB - 1) // GB) * GB
    S = T * W
    NB = T // GB

    # global slot ids
    slot_of = np.full(N, -1, np.int64)
    for k in range(NCORES):
        for t, nl in enumerate(tiles_pc[k]):
            slot_of[nl + k * NPC] = k * S + t * W + np.arange(len(nl))
    assert (slot_of >= 0).all()

    w_bf = {}
    for nm in ("Wl1", "Wr1", "Wl2", "Wr2"):
        w_bf[nm] = np.asarray(inputs[nm], np.float32).astype(BF16)
    rep = lambda v: np.repeat(np.asarray(v, np.float32)[None, :], P, 0).astype(BF16)
    repT = lambda v: np.broadcast_to(
        np.asarray(v, np.float32).reshape(2, P).transpose(1, 0)[:, :, None],
        (P, 2, P)).astype(BF16).copy()
    att1r = rep(np.asarray(inputs["att1"], np.float32).reshape(HD))
    att2r = rep(np.asarray(inputs["att2"], np.float32).reshape(HD))
    bl1r, br1r = rep(inputs["bl1"]), rep(inputs["br1"])
    bl2r, br2r = rep(inputs["bl2"]), rep(inputs["br2"])
    bias1T, bias2T = repT(inputs["bias1"]), repT(inputs["bias2"])

    in_maps = []
    for k in range(NCORES):
        s_k, d_k, counts = per_core[k]
        tiles = tiles_pc[k]
        starts = np.zeros(NPC + 1, np.int64)
        np.cumsum(counts, out=starts[1:])

        xT = np.zeros((IN, S), np.float32)
        srcidx = np.zeros((P, T), np.int32)
        i0 = np.zeros((P, T, W), np.float32)
        i0t = np.zeros((W, T, P), np.float32)
        gind = np.zeros((P, NB, G), np.float32)
        for t, nl in enumerate(tiles):
            nw = len(nl)
            nodes = nl + k * NPC
            xT[:, t * W:t * W + nw] = x[nodes].T
            gslot = t * W + np.arange(nw)          # local slot of window rows
            brow, prow = divmod(gslot, P)          # batch id / row within
            gind[prow, brow, batch[nodes]] = inv_cnt[batch[nodes]]
            tile_cnt = counts[nl]
            srcs = np.concatenate(
                [s_k[starts[n]:starts[n + 1]] for n in nl]) if nw else                 np.zeros(0, np.int64)
            ke = int(tile_cnt.sum())
            if ke:
                srcidx[:ke, t] = slot_of[srcs]
                offs = np.repeat(np.arange(nw), tile_cnt)
                i0[np.arange(ke), t, offs] = 1.0
                i0t[offs, t, np.arange(ke)] = 1.0

        in_maps.append({
            "xT": xT.astype(BF16),
            "wl1": w_bf["Wl1"], "wr1": w_bf["Wr1"],
            "wl2": w_bf["Wl2"], "wr2": w_bf["Wr2"],
            "bl1r": bl1r, "br1r": br1r, "bl2r": bl2r, "br2r": br2r,
            "bias1T": bias1T.reshape(P, 2 * P), "bias2T": bias2T.reshape(P, 2 * P),
            "att1r": att1r, "att2r": att2r,
            "srcidx": srcidx,
            "i0": i0.reshape(P, T * W).astype(BF16),
            "i0t": i0t.reshape(W, T * P).astype(BF16),
            "gind": gind.reshape(P, NB * G).astype(BF16),
        })
    return in_maps, T

# ------------------------------------------------------------- bass program


def _legalize_waits(nc):
    """walrus allows 1 sync wait on DMA/CTRL instrs, 2 on compute instrs.
    Hoist excess waits onto same-engine NoOps inserted just before."""
    n_ins = 0
    for blk in nc.m.functions[0].blocks:
        out = []
        for inst in blk.instructions:
            si = inst.sync_info
            waits = list(si.on_wait) if (si is not None and si.on_wait) else []
            lim = 1
            if len(waits) > lim:
                for wchunk in waits[:-lim]:
                    nop = mybir.InstNoOp(name=f"waitnop_{n_ins}_{inst.name}",
                                         ins=[], outs=[])
                    nop.engine = inst.engine
                    nop.sync_info = mybir.SyncInfo(on_wait=[wchunk],
                                                   on_update=[])
                    out.append(nop)
                    n_ins += 1
                si.on_wait = waits[-lim:]
            out.append(inst)
        blk.instructions = out
    return n_ins


def _bc_mid(ap, axis, n):
    """Insert a stride-0 broadcast dim of size n at `axis` into an AP."""
    lst = [list(d) for d in ap.ap]
    lst.insert(axis, [0, n])
    return bass.AP(ap.tensor, ap.offset, lst)


def _transform_chunk(nc, psum, pool, lhs_chunks, w_sb, nk, brep, dst_rows,
                     tag):
    """dst_rows <- (lhsT.T @ w) + bias_rep, bf16."""
    ps = psum.tile([P, HD], f32, tag="ps_tf", name=f"ps_{tag}")
    for kk in range(nk):
        nc.tensor.matmul(ps[:], lhsT=lhs_chunks[kk], rhs=w_sb[:, kk, :],
                         start=(kk == 0), stop=(kk == nk - 1))
    o0 = pool.tile([P, HD], bf, tag=f"o0_{tag}")
    nc.scalar.activation(o0[:], ps[:], mybir.ActivationFunctionType.Copy)
    o1 = pool.tile([P, HD], bf, tag=f"o1_{tag}")
    nc.vector.tensor_tensor(out=o1[:], in0=o0[:], in1=brep[:],
                            op=mybir.AluOpType.add)
    nc.sync.dma_start(dst_rows, o1[:])


def _edge_phase(nc, tc, ctx, T, layer, xl_full, xr_slot, d_in, ident_bf,
                consts, xl2_slot=None, xr2_slot=None, pool_ps=None,
                h2T_dram=None):
    NB = T // GB
    S_rows = T * W
    pool = ctx.enter_context(tc.tile_pool(name=f"e{layer}_sb", bufs=2))
    gpool = ctx.enter_context(tc.tile_pool(name=f"e{layer}_g", bufs=4))
    psS = ctx.enter_context(tc.tile_pool(name=f"e{layer}_psS", bufs=2,
                                         space="PSUM"))
    psN = ctx.enter_context(tc.tile_pool(name=f"e{layer}_psN", bufs=2,
                                         space="PSUM"))
    psD = psN
    att_rep = consts["att1r" if layer == 1 else "att2r"]
    biasT = consts["bias1T" if layer == 1 else "bias2T"]

    Act = mybir.ActivationFunctionType
    for b in range(NB):
        t0 = b * GB
        sidx = gpool.tile([P, GB], i32, tag="sidx")
        nc.sync.dma_start(sidx[:], d_in["srcidx"][:, t0:t0 + GB])
        i0b = gpool.tile([P, GB, W], bf, tag="i0b")
        nc.sync.dma_start(i0b[:], d_in["i0"][:, t0 * W:(t0 + GB) * W])
        i0tb = gpool.tile([W, GB, P], bf, tag="i0tb")
        nc.sync.dma_start(i0tb[:], d_in["i0t"][:, t0 * P:(t0 + GB) * P])
        # window rows in window-major layout: partition w, free (j, c)
        xr_t = gpool.tile([W, GB, HD], bf, tag="xr")
        nc.sync.dma_start(
            xr_t[:],
            xr_slot[b * P:(b + 1) * P, :].rearrange("(j w) c -> w j c", w=W))
        if layer == 2:
            gind_b = gpool.tile([P, G], bf, tag="gind")
            nc.sync.dma_start(gind_b[:], d_in["gind"][:, b * G:(b + 1) * G])
        xl_g = gpool.tile([P, GB, HD], bf, tag="xlg")
        for j in range(GB):
            nc.gpsimd.indirect_dma_start(
                out=xl_g[:, j, :], out_offset=None, in_=xl_full[:, :],
                in_offset=bass.IndirectOffsetOnAxis(
                    ap=sidx[:, j:j + 1], axis=0))

        # scores: S = xr[dst] + xl[src]; m = leaky(S)
        m_all = pool.tile([P, GB, HD], bf, tag="m")
        for j in range(GB):
            S_ps = psS.tile([P, HD], f32, tag="S")
            nc.tensor.matmul(S_ps[:], lhsT=i0tb[:, j, :],
                             rhs=xr_t[:, j, :],
                             start=True, stop=False)
            nc.tensor.matmul(S_ps[:], lhsT=ident_bf[:], rhs=xl_g[:, j, :],
                             start=False, stop=True)
            nc.scalar.activation(m_all[:, j, :], S_ps[:], Act.Prelu,
                                 alpha=NEG)
        wm = pool.tile([P, GB, H, D], bf, tag="wm")
        nc.vector.tensor_tensor(out=wm[:],
                                in0=m_all[:].rearrange(
                                    "p g (h d) -> p g h d", h=H),
                                in1=_bc_mid(att_rep[:], 1, GB).rearrange(
                                    "p g (h d) -> p g h d", h=H),
                                op=mybir.AluOpType.mult)
        # binary-tree reduce over D=64 within each head
        tprev = wm
        half = D
        while half > 1:
            half //= 2
            tn = pool.tile([P, GB, H, half], bf, tag=f"tr{half}")
            nc.vector.tensor_tensor(out=tn[:], in0=tprev[:, :, :, 0:half],
                                    in1=tprev[:, :, :, half:2 * half],
                                    op=mybir.AluOpType.add)
            tprev = tn
        p_all = pool.tile([P, GB, H], bf, tag="p")
        nc.scalar.activation(p_all[:], tprev[:, :, :, 0], Act.Exp)

        denre = psD.tile([P, GB, 2 * H], f32, tag="denre")
        den_t = denre[0:W, :, 0:H]
        re_ps = denre[:, :, H:2 * H]
        for j in range(GB):
            nc.tensor.matmul(den_t[:, j, :],
                             lhsT=i0b[:, j, :], rhs=p_all[:, j, :],
                             start=True, stop=True)
        dens = pool.tile([W, GB, H], f32, tag="dens")
        nc.vector.tensor_scalar_add(dens[:], den_t[:], 1e-16)
        recip = pool.tile([W, GB, H], bf, tag="recip")
        with nc.allow_low_precision(reason="attn denom O(1)"):
            nc.vector.reciprocal(recip[:], dens[:])
        for j in range(GB):
            nc.tensor.matmul(re_ps[:, j, :], lhsT=i0tb[:, j, :],
                             rhs=recip[:, j, :],
                             start=True, stop=True)
        alpha = pool.tile([P, GB, H], bf, tag="alpha")
        nc.vector.tensor_tensor(out=alpha[:], in0=p_all[:], in1=re_ps[:],
                                op=mybir.AluOpType.mult)
        # fold alpha into indicator: i0a[p,g,h,w] = i0[p,g,w]*alpha[p,g,h]
        i0a = pool.tile([P, GB, H, W], bf, tag="i0a")
        nc.vector.tensor_tensor(out=i0a[:], in0=_bc_mid(i0b[:], 2, H),
                                in1=alpha[:].to_broadcast([P, GB, H, W]),
                                op=mybir.AluOpType.mult)
        # transposed aggregation: hT[f, j, c, w] = sum_e alpha*xl, f=64h+d
        hT_ps = psN.tile([P, GB, 2, W], f32, tag="hT")
        for j in range(GB):
            for h in range(H):
                nc.tensor.matmul(
                    hT_ps[64 * (h % 2):64 * (h % 2) + 64, j, h // 2, :],
                    lhsT=xl_g[:, j, D * h:D * (h + 1)],
                    rhs=i0a[:, j, h, :],
                    start=True, stop=True)
        hbT = pool.tile([P, 2, GB * W], bf, tag="hbT")
        for c in range(2):
            nsbT = pool.tile([P, GB, W], bf, tag=f"nsbT{c}",
                             name=f"nsbT{c}")
            nc.scalar.activation(nsbT[:], hT_ps[:, :, c, :], Act.Copy)
            nc.vector.tensor_tensor(out=hbT[:, c, :], in0=nsbT[:].rearrange(
                                        "p j w -> p (j w)"),
                                    in1=biasT[:, c, :],
                                    op=mybir.AluOpType.add)

        if layer == 1:
            # silu(x) = x / (1 + exp(-x)), in transposed layout
            hT_sb = pool.tile([P, 2, GB * W], bf, tag="hTs")
            for c in range(2):
                sg = pool.tile([P, GB * W], bf, tag=f"sg{c}")
                nc.scalar.activation(sg[:], hbT[:, c, :], Act.Exp,
                                     scale=-1.0)
                d2 = pool.tile([P, GB * W], f32, tag=f"d2{c}")
                nc.vector.tensor_scalar_add(d2[:], sg[:], 1.0)
                rc = pool.tile([P, GB * W], bf, tag=f"rc{c}")
                with nc.allow_low_precision(reason="silu denom in [1,2]"):
                    nc.vector.reciprocal(rc[:], d2[:])
                nc.vector.tensor_tensor(out=hT_sb[:, c, :],
                                        in0=hbT[:, c, :], in1=rc[:],
                                        op=mybir.AluOpType.mult)
            # fused layer-2 transforms: hT_sb chunks ARE the lhsT
            lhs = [hT_sb[:, 0, :], hT_sb[:, 1, :]]
            _transform_chunk(nc, psN, pool, lhs, consts["wl2_sb"], 2,
                             consts["bl2r"],
                             xl2_slot[b * P:(b + 1) * P, :], "xl2")
            _transform_chunk(nc, psN, pool, lhs, consts["wr2_sb"], 2,
                             consts["br2r"],
                             xr2_slot[b * P:(b + 1) * P, :], "xr2")
        else:
            # stage h2^T to DRAM, read back transposed, pool immediately
            for c in range(2):
                nc.sync.dma_start(
                    h2T_dram[c * P:(c + 1) * P, b * P:(b + 1) * P],
                    hbT[:, c, :])
                h2c = pool.tile([P, P], bf, tag=f"h2c{c}", name=f"h2c{c}")
                nc.sync.dma_start(
                    h2c[:],
                    h2T_dram[c * P:(c + 1) * P, b * P:(b + 1) * P],
                    transpose=True)
                nc.tensor.matmul(pool_ps[c][:],
                                 lhsT=gind_b[:], rhs=h2c[:],
                                 start=(b == 0), stop=(b == NB - 1))


def build_program(T):
    nc = bass.Bass()
    S = T * W
    NB = T // GB
    d_in = {}
    for name, shape, dt in [
        ("xT", [IN, S], bf),
        ("wl1", [IN, HD], bf), ("wr1", [IN, HD], bf),
        ("wl2", [HD, HD], bf), ("wr2", [HD, HD], bf),
        ("bl1r", [P, HD], bf), ("br1r", [P, HD], bf),
        ("bl2r", [P, HD], bf), ("br2r", [P, HD], bf),
        ("bias1T", [P, 2 * P], bf), ("bias2T", [P, 2 * P], bf),
        ("att1r", [P, HD], bf), ("att2r", [P, HD], bf),
        ("srcidx", [P, T], i32),
        ("i0", [P, T * W], bf), ("i0t", [W, T * P], bf),
        ("gind", [P, NB * G], bf),
    ]:
        d_in[name] = nc.declare_dram_parameter(name, shape, dt,
                                               isOutput=False)
    out = nc.declare_dram_parameter("out", [G, HD], f32, isOutput=True)
    dbg = os.environ.get("GAT_DEBUG", "0") == "1"
    if dbg:
        dbg_xl1 = nc.declare_dram_parameter("dbg_xl1", [S, HD], bf,
                                            isOutput=True)
        dbg_xl2 = nc.declare_dram_parameter("dbg_xl2", [S, HD], bf,
                                            isOutput=True)
        dbg_pool = nc.declare_dram_parameter("dbg_pool", [G, HD], f32,
                                             isOutput=True)
        dbg_h2T = nc.declare_dram_parameter("dbg_h2T", [2 * P, S], bf,
                                            isOutput=True)

    xl1_slot = nc.dram_tensor("xl1_slot", [S, HD], bf)
    xr1_slot = nc.dram_tensor("xr1_slot", [S, HD], bf)
    xl1_full = nc.dram_tensor("xl1_full", [NCORES * S, HD], bf,
                              addr_space="Shared")
    xl2_slot = nc.dram_tensor("xl2_slot", [S, HD], bf)
    xr2_slot = nc.dram_tensor("xr2_slot", [S, HD], bf)
    xl2_full = nc.dram_tensor("xl2_full", [NCORES * S, HD], bf,
                              addr_space="Shared")
    h2T_dram = nc.dram_tensor("h2T_dram", [2 * P, S], bf)
    pool_loc = nc.dram_tensor("pool_loc", [G, HD], f32)
    pool_sum = nc.dram_tensor("pool_sum", [G, HD], f32, addr_space="Shared")

    with tile.TileContext(nc) as tc, ExitStack() as ctx:
        from concourse.masks import make_identity
        gc = ctx.enter_context(tc.tile_pool(name="gc", bufs=1))
        ident = gc.tile([P, P], f32)
        make_identity(nc, ident[:])
        ident_bf = gc.tile([P, P], bf)
        nc.vector.tensor_copy(ident_bf[:], ident[:])

        consts = {}
        for nm in ("att1r", "att2r", "bl1r", "br1r", "bl2r", "br2r"):
            t = gc.tile([P, HD], bf, tag=nm, name=f"c_{nm}")
            nc.sync.dma_start(t[:], d_in[nm][:, :])
            consts[nm] = t
        for nm in ("bias1T", "bias2T"):
            t = gc.tile([P, 2, P], bf, tag=nm, name=f"c_{nm}")
            nc.sync.dma_start(t[:].rearrange("p c k -> p (c k)"),
                              d_in[nm][:, :])
            consts[nm] = t
        for nm, src in (("wl2_sb", "wl2"), ("wr2_sb", "wr2")):
            t = gc.tile([P, 2, HD], bf, tag=nm, name=f"c_{nm}")
            for kk in range(2):
                nc.sync.dma_start(t[:, kk, :],
                                  d_in[src][kk * P:(kk + 1) * P, :])
            consts[nm] = t

        # phase A: layer-1 transforms over slot chunks
        with ExitStack() as c1:
            tp = c1.enter_context(tc.tile_pool(name="tfA", bufs=2))
            tps = c1.enter_context(tc.tile_pool(name="tfA_ps", bufs=2,
                                                space="PSUM"))
            cw = c1.enter_context(tc.tile_pool(name="tfA_c", bufs=1))
            xT_sb = cw.tile([IN, S], bf)
            nc.sync.dma_start(xT_sb[:], d_in["xT"][:, :])
            wl1_sb = cw.tile([IN, 1, HD], bf)
            nc.sync.dma_start(wl1_sb[:, 0, :], d_in["wl1"][:, :])
            wr1_sb = cw.tile([IN, 1, HD], bf)
            nc.sync.dma_start(wr1_sb[:, 0, :], d_in["wr1"][:, :])
            for c in range(NB):
                lhs = [xT_sb[:, c * P:(c + 1) * P]]
                _transform_chunk(nc, tps, tp, lhs, wl1_sb, 1,
                                 consts["bl1r"],
                                 xl1_slot[c * P:(c + 1) * P, :], "xl1")
                _transform_chunk(nc, tps, tp, lhs, wr1_sb, 1,
                                 consts["br1r"],
                                 xr1_slot[c * P:(c + 1) * P, :], "xr1")

        # phase B: AllGather xl1
        nc.gpsimd.collective_compute(
            "AllGather", mybir.AluOpType.bypass,
            replica_groups=[list(range(NCORES))],
            ins=[xl1_slot[0:S, :]], outs=[xl1_full[:, :]])

        # phase C: edge layer 1 (+ fused layer-2 transforms)
        with ExitStack() as c2:
            _edge_phase(nc, tc, c2, T, 1, xl1_full, xr1_slot, d_in,
                        ident_bf, consts, xl2_slot=xl2_slot,
                        xr2_slot=xr2_slot)

        # phase D: AllGather xl2
        nc.gpsimd.collective_compute(
            "AllGather", mybir.AluOpType.bypass,
            replica_groups=[list(range(NCORES))],
            ins=[xl2_slot[0:S, :]], outs=[xl2_full[:, :]])

        # phase E: edge layer 2 + pool accumulation
        with ExitStack() as c3:
            plp = c3.enter_context(tc.tile_pool(name="poolps", bufs=1,
                                                space="PSUM"))
            pool_ps = [plp.tile([G, P], f32, tag=f"pps{c}", name=f"pps{c}")
                       for c in range(2)]
            _edge_phase(nc, tc, c3, T, 2, xl2_full, xr2_slot, d_in,
                        ident_bf, consts, pool_ps=pool_ps,
                        h2T_dram=h2T_dram)
            psb = c3.enter_context(tc.tile_pool(name="poolsb", bufs=1))
            pool_sb = psb.tile([G, HD], f32)
            for c in range(2):
                nc.scalar.activation(pool_sb[:, P * c:P * (c + 1)],
                                     pool_ps[c][:],
                                     mybir.ActivationFunctionType.Copy)
            nc.sync.dma_start(pool_loc[:, :], pool_sb[:])
            if dbg:
                nc.sync.dma_start(dbg_pool[:, :], pool_sb[:])
                dpool = c3.enter_context(tc.tile_pool(name="dbgp", bufs=2))
                for c in range(NB):
                    dt1 = dpool.tile([P, HD], bf, tag="d1")
                    nc.sync.dma_start(dt1[:], xl1_slot[c * P:(c + 1) * P, :])
                    nc.sync.dma_start(dbg_xl1[c * P:(c + 1) * P, :], dt1[:])
                    dt2 = dpool.tile([P, HD], bf, tag="d2")
                    nc.sync.dma_start(dt2[:], xl2_slot[c * P:(c + 1) * P, :])
                    nc.sync.dma_start(dbg_xl2[c * P:(c + 1) * P, :], dt2[:])
                    for cc in range(2):
                        dt3 = dpool.tile([P, P], bf, tag="d3", name="dt3")
                        nc.sync.dma_start(
                            dt3[:],
                            h2T_dram[cc * P:(cc + 1) * P,
                                     c * P:(c + 1) * P])
                        nc.sync.dma_start(
                            dbg_h2T[cc * P:(cc + 1) * P,
                                    c * P:(c + 1) * P], dt3[:])
            nc.gpsimd.collective_compute(
                "AllReduce", mybir.AluOpType.add,
                replica_groups=[list(range(NCORES))],
                ins=[pool_loc[:, :]], outs=[pool_sum[:, :]])
            outt = psb.tile([G, HD], f32)
            nc.sync.dma_start(outt[:], pool_sum[:, :])
            nc.sync.dma_start(out[:, :], outt[:])

    return nc


# ------------------------------------------------------------------- driver


def _pjrt_prepare(nc, in_maps):
    """Build the jitted 8-core executable + device-resident inputs."""
    import jax
    from jax.sharding import Mesh, PartitionSpec
    from jax.experimental.shard_map import shard_map
    from concourse import bass2jax

    bass2jax.install_neuronx_cc_hook()
    n_cores = len(in_maps)
    partition_name = (nc.partition_id_tensor.name
                      if nc.partition_id_tensor else None)
    in_names, out_names, out_avals, zero_outs = [], [], [], []
    for alloc in nc.m.functions[0].allocations:
        if not isinstance(alloc, mybir.MemoryLocationSet):
            continue
        name = alloc.memorylocations[0].name
        if alloc.kind == "ExternalInput":
            if name != partition_name:
                in_names.append(name)
        elif alloc.kind == "ExternalOutput":
            out_names.append(name)
            shape = tuple(alloc.tensor_shape)
            dtype = mybir.dt.np(alloc.dtype)
            out_avals.append(jax.core.ShapedArray(shape, dtype))
            zero_outs.append(np.zeros(shape, dtype))
    n_params = len(in_names)
    n_outs = len(out_avals)
    all_in_names = list(in_names) + list(out_names)
    if partition_name is not None:
        all_in_names.append(partition_name)

    def _body(*args):
        operands = list(args)
        if partition_name is not None:
            operands.append(bass2jax.partition_id_tensor())
        outs = bass2jax._bass_exec_p.bind(
            *operands,
            out_avals=tuple(out_avals),
            in_names=tuple(all_in_names),
            out_names=tuple(out_names),
            lowering_input_output_aliases=(),
            sim_require_finite=True,
            sim_require_nnan=True,
            nc=nc,
        )
        return tuple(outs)

    devices = jax.devices()[:n_cores]
    mesh = Mesh(np.asarray(devices), ("core",))
    in_specs = (PartitionSpec("core"),) * (n_params + n_outs)
    out_specs = (PartitionSpec("core"),) * len(out_names)
    sharded = jax.jit(
        shard_map(_body, mesh=mesh, in_specs=in_specs, out_specs=out_specs,
                  check_rep=False),
        keep_unused=True)
    concat_in = [
        np.concatenate([np.asarray(in_maps[c][nm]) for c in range(n_cores)],
                       axis=0)
        for nm in in_names
    ]
    from jax.sharding import NamedSharding
    sh = NamedSharding(mesh, PartitionSpec("core"))
    dev_in = [jax.device_put(a, sh) for a in concat_in]

    dev_zeros = [jax.device_put(
        np.zeros((n_cores * z.shape[0], *z.shape[1:]), z.dtype), sh)
        for z in zero_outs]

    def run_fn():
        outs = sharded(*dev_in, *dev_zeros)
        import jax as _j
        _j.block_until_ready(outs)
        return outs

    return run_fn, out_names, out_avals


def kernel(**inputs):
    in_maps, T = _host_prep(inputs)
    nc = build_program(T)
    _legalize_waits(nc)
    run_fn, out_names, out_avals = _pjrt_prepare(nc, in_maps)
    outs = run_fn()   # compile + first exec
    if os.environ.get("GAT_BENCH", "0") == "1":
        import time
        times = []
        for _ in range(int(os.environ.get("GAT_BENCH_ITERS", "5"))):
            t0 = time.perf_counter()
            outs = run_fn()
            times.append(time.perf_counter() - t0)
        kernel.last_exec_time_ns = int(min(times) * 1e9)
        kernel.bench_times = times
    i = out_names.index("out")
    full = np.asarray(outs[i]).reshape(NCORES, *out_avals[i].shape)
    return np.asarray(full[0], np.float32)


# revision 43
# speedup vs baseline: 1.1836x; 1.1836x over previous
"""GATv2 x2 + global mean pool on 8 Trainium2 NeuronCores (Bass/Tile), v2.

Slot-space layout (dst-sharded, uniform tiles):
  - Edges sorted by dst; per core, greedy tiles of <=8 dst nodes ("window
    slots") and <=128 edges. Tile t owns slot rows [8t, 8t+8); a batch of
    16 tiles = 128 consecutive slot rows, so ALL window-side accesses
    (xr rows, pool indicators) are plain contiguous DMAs.
  - Node tables (xl/xr) are computed in slot space; xl tables are
    AllGathered to a [8*S, 256] global table; per-edge xl rows come via
    per-tile indirect DMA gathers with global-slot indices.
  - Scores: tables have no aux cols; e = sum_hd att*leaky(S) computed as
    Prelu(S) on ACT, att-mul + binary-tree head reduction on DVE.
  - Aggregation: alpha is folded into the one-hot indicator (i0a = i0 *
    alpha per head), so num = sum per head via 4 matmuls per tile and the
    per-edge [128,256] alpha-broadcast multiply disappears.
  - silu via exp table (1/(1+e^-x)*x) to keep a single ACT table set
    (prelu/exp/copy) with zero table reloads.
  - Layer-2 transforms are fused into the layer-1 edge loop: h tiles are
    PE-transposed in SBUF and immediately transformed; h never goes to
    DRAM.
  - Global mean pool via per-batch [128,64] indicator matmul accumulated
    in PSUM across all batches; final AllReduce over [64,256].
"""
import sys

sys.path.insert(0, "/opt/trn_rl_repo")
sys.path.insert(0, "/opt/pypackages")

import os
from contextlib import ExitStack

import numpy as np
import ml_dtypes

import concourse.bass as bass
import concourse.mybir as mybir
import concourse.tile as tile

BF16 = ml_dtypes.bfloat16
bf = mybir.dt.bfloat16
f32 = mybir.dt.float32
i32 = mybir.dt.int32

N, E, G = 50000, 800000, 64
IN, H, D = 128, 4, 64
HD = H * D
NCORES = 8
NPC = N // NCORES
P = 128                      # edge slots per tile
W = 8                        # window (dst-node) slots per tile
GB = 16                      # tiles per batch (= 128 slot rows)
NEG = 0.2

# ---------------------------------------------------------------- host prep


def _tile_core(dst_l):
    """Bin-pack local nodes into tiles of <=W nodes and <=P edges.

    Nodes are relabeled freely within the core (everything downstream is
    slot-indexed). Snake-deal by degree, then repair overloaded bins.
    Returns (tiles, counts): tiles = list of node-id arrays.
    """
    counts = np.bincount(dst_l, minlength=NPC)
    assert counts.max() <= P, counts.max()
    order = np.argsort(-counts, kind="stable")
    import heapq
    nb = max((NPC + W - 1) // W, int(np.ceil(counts.sum() / P)))
    while True:
        bins = [[] for _ in range(nb)]
        load = np.zeros(nb, np.int64)
        heap = [(0, b) for b in range(nb)]
        heapq.heapify(heap)
        ok = True
        for n in order:
            c = int(counts[n])
            parked = []
            placed = False
            while heap:
                ld, b = heapq.heappop(heap)
                if ld + c <= P and len(bins[b]) < W:
                    bins[b].append(n)
                    load[b] = ld + c
                    if len(bins[b]) < W:
                        heapq.heappush(heap, (ld + c, b))
                    placed = True
                    break
                parked.append((ld, b))
                if ld + c > P:
                    break
            for it in parked:
                heapq.heappush(heap, it)
            if not placed:
                ok = False
                break
        if ok:
            return [np.sort(np.asarray(b, np.int64)) for b in bins], counts
        nb += 4


def _host_prep(inputs):
    x = np.asarray(inputs["x"], np.float32)
    ei = np.asarray(inputs["edge_index"]).astype(np.int64)
    batch = np.asarray(inputs["batch"]).astype(np.int64)

    src, dst = ei[0], ei[1]
    order = np.argsort(dst, kind="stable")
    src_s, dst_s = src[order].astype(np.int64), dst[order].astype(np.int64)
    core_of = dst_s // NPC

    cnt = np.bincount(batch, minlength=G).astype(np.float32)
    inv_cnt = (1.0 / np.maximum(cnt, 1.0)).astype(np.float32)

    per_core = []
    tiles_pc = []
    for k in range(NCORES):
        m = core_of == k
        s_k, d_k = src_s[m], dst_s[m] - k * NPC
        tiles, counts = _tile_core(d_k)
        per_core.append((s_k, d_k, counts))
        tiles_pc.append(tiles)

    T = max(len(t) for t in tiles_pc)
    T = ((T + GB - 1) // GB) * GB
    S = T * W
    NB = T // GB

    # global slot ids
    slot_of = np.full(N, -1, np.int64)
    for k in range(NCORES):
        for t, nl in enumerate(tiles_pc[k]):
            slot_of[nl + k * NPC] = k * S + t * W + np.arange(len(nl))
    assert (slot_of >= 0).all()

    w_bf = {}
    for nm in ("Wl1", "Wr1", "Wl2", "Wr2"):
        w_bf[nm] = np.asarray(inputs[nm], np.float32).astype(BF16)
    rep = lambda v: np.repeat(np.asarray(v, np.float32)[None, :], P, 0).astype(BF16)
    repT = lambda v: np.broadcast_to(
        np.asarray(v, np.float32).reshape(2, P).transpose(1, 0)[:, :, None],
        (P, 2, P)).astype(BF16).copy()
    att1r = rep(np.asarray(inputs["att1"], np.float32).reshape(HD))
    att2r = rep(np.asarray(inputs["att2"], np.float32).reshape(HD))
    bl1r, br1r = rep(inputs["bl1"]), rep(inputs["br1"])
    bl2r, br2r = rep(inputs["bl2"]), rep(inputs["br2"])
    bias1T, bias2T = repT(inputs["bias1"]), repT(inputs["bias2"])

    in_maps = []
    for k in range(NCORES):
        s_k, d_k, counts = per_core[k]
        tiles = tiles_pc[k]
        starts = np.zeros(NPC + 1, np.int64)
        np.cumsum(counts, out=starts[1:])

        xT = np.zeros((IN, S), np.float32)
        srcidx = np.zeros((P, T), np.int32)
        i0 = np.zeros((P, T, W), np.float32)
        i0t = np.zeros((W, T, P), np.float32)
        gind = np.zeros((P, NB, G), np.float32)
        for t, nl in enumerate(tiles):
            nw = len(nl)
            nodes = nl + k * NPC
            xT[:, t * W:t * W + nw] = x[nodes].T
            gslot = t * W + np.arange(nw)          # local slot of window rows
            brow, prow = divmod(gslot, P)          # batch id / row within
            gind[prow, brow, batch[nodes]] = inv_cnt[batch[nodes]]
            tile_cnt = counts[nl]
            srcs = np.concatenate(
                [s_k[starts[n]:starts[n + 1]] for n in nl]) if nw else                 np.zeros(0, np.int64)
            ke = int(tile_cnt.sum())
            if ke:
                srcidx[:ke, t] = slot_of[srcs]
                offs = np.repeat(np.arange(nw), tile_cnt)
                i0[np.arange(ke), t, offs] = 1.0
                i0t[offs, t, np.arange(ke)] = 1.0

        in_maps.append({
            "xT": xT.astype(BF16),
            "wl1": w_bf["Wl1"], "wr1": w_bf["Wr1"],
            "wl2": w_bf["Wl2"], "wr2": w_bf["Wr2"],
            "bl1r": bl1r, "br1r": br1r, "bl2r": bl2r, "br2r": br2r,
            "bias1T": bias1T.reshape(P, 2 * P), "bias2T": bias2T.reshape(P, 2 * P),
            "att1r": att1r, "att2r": att2r,
            "srcidx": srcidx,
            "i0": i0.reshape(P, T * W).astype(BF16),
            "i0t": i0t.reshape(W, T * P).astype(BF16),
            "gind": gind.reshape(P, NB * G).astype(BF16),
        })
    return in_maps, T

# ------------------------------------------------------------- bass program


def _legalize_waits(nc):
    """walrus allows 1 sync wait on DMA/CTRL instrs, 2 on compute instrs.
    Hoist excess waits onto same-engine NoOps inserted just before."""
    n_ins = 0
    for blk in nc.m.functions[0].blocks:
        out = []
        for inst in blk.instructions:
            si = inst.sync_info
            waits = list(si.on_wait) if (si is not None and si.on_wait) else []
            lim = 1
            if len(waits) > lim:
                for wchunk in waits[:-lim]:
                    nop = mybir.InstNoOp(name=f"waitnop_{n_ins}_{inst.name}",
                                         ins=[], outs=[])
                    nop.engine = inst.engine
                    nop.sync_info = mybir.SyncInfo(on_wait=[wchunk],
                                                   on_update=[])
                    out.append(nop)
                    n_ins += 1
                si.on_wait = waits[-lim:]
            out.append(inst)
        blk.instructions = out
    return n_ins


def _bc_mid(ap, axis, n):
    """Insert a stride-0 broadcast dim of size n at `axis` into an AP."""
    lst = [list(d) for d in ap.ap]
    lst.insert(axis, [0, n])
    return bass.AP(ap.tensor, ap.offset, lst)


def _transform_chunk(nc, psum, pool, lhs_chunks, w_sb, nk, brep, dst_rows,
                     tag):
    """dst_rows <- (lhsT.T @ w) + bias_rep, bf16."""
    ps = psum.tile([P, HD], f32, tag="ps_tf", name=f"ps_{tag}")
    for kk in range(nk):
        nc.tensor.matmul(ps[:], lhsT=lhs_chunks[kk], rhs=w_sb[:, kk, :],
                         start=(kk == 0), stop=(kk == nk - 1))
    o0 = pool.tile([P, HD], bf, tag=f"o0_{tag}")
    nc.scalar.activation(o0[:], ps[:], mybir.ActivationFunctionType.Copy)
    o1 = pool.tile([P, HD], bf, tag=f"o1_{tag}")
    nc.vector.tensor_tensor(out=o1[:], in0=o0[:], in1=brep[:],
                            op=mybir.AluOpType.add)
    nc.sync.dma_start(dst_rows, o1[:])


def _edge_phase(nc, tc, ctx, T, layer, xl_full, xr_slot, d_in, ident_bf,
                consts, xl2_slot=None, xr2_slot=None, pool_ps=None,
                h2T_dram=None):
    NB = T // GB
    S_rows = T * W
    pool = ctx.enter_context(tc.tile_pool(name=f"e{layer}_sb", bufs=2))
    gpool = ctx.enter_context(tc.tile_pool(name=f"e{layer}_g", bufs=4))
    psS = ctx.enter_context(tc.tile_pool(name=f"e{layer}_psS", bufs=2,
                                         space="PSUM"))
    psN = ctx.enter_context(tc.tile_pool(name=f"e{layer}_psN", bufs=2,
                                         space="PSUM"))
    psD = psN
    att_rep = consts["att1r" if layer == 1 else "att2r"]
    biasT = consts["bias1T" if layer == 1 else "bias2T"]

    Act = mybir.ActivationFunctionType
    for b in range(NB):
        t0 = b * GB
        sidx = gpool.tile([P, GB], i32, tag="sidx")
        nc.sync.dma_start(sidx[:], d_in["srcidx"][:, t0:t0 + GB])
        i0b = gpool.tile([P, GB, W], bf, tag="i0b")
        nc.sync.dma_start(i0b[:], d_in["i0"][:, t0 * W:(t0 + GB) * W])
        i0tb = gpool.tile([W, GB, P], bf, tag="i0tb")
        nc.sync.dma_start(i0tb[:], d_in["i0t"][:, t0 * P:(t0 + GB) * P])
        # window rows in window-major layout: partition w, free (j, c)
        xr_t = gpool.tile([W, GB, HD], bf, tag="xr")
        nc.sync.dma_start(
            xr_t[:],
            xr_slot[b * P:(b + 1) * P, :].rearrange("(j w) c -> w j c", w=W))
        if layer == 2:
            gind_b = gpool.tile([P, G], bf, tag="gind")
            nc.sync.dma_start(gind_b[:], d_in["gind"][:, b * G:(b + 1) * G])
        xl_g = gpool.tile([P, GB, HD], bf, tag="xlg")
        for j in range(GB):
            nc.gpsimd.indirect_dma_start(
                out=xl_g[:, j, :], out_offset=None, in_=xl_full[:, :],
                in_offset=bass.IndirectOffsetOnAxis(
                    ap=sidx[:, j:j + 1], axis=0))

        # scores: S = xr[dst] + xl[src]; m = leaky(S)
        m_all = pool.tile([P, GB, HD], bf, tag="m")
        for j in range(GB):
            S_ps = psS.tile([P, HD], f32, tag="S")
            nc.tensor.matmul(S_ps[:], lhsT=i0tb[:, j, :],
                             rhs=xr_t[:, j, :],
                             start=True, stop=False)
            nc.tensor.matmul(S_ps[:], lhsT=ident_bf[:], rhs=xl_g[:, j, :],
                             start=False, stop=True)
            nc.scalar.activation(m_all[:, j, :], S_ps[:], Act.Prelu,
                                 alpha=NEG)
        wm = pool.tile([P, GB, H, D], bf, tag="wm")
        nc.vector.tensor_tensor(out=wm[:],
                                in0=m_all[:].rearrange(
                                    "p g (h d) -> p g h d", h=H),
                                in1=_bc_mid(att_rep[:], 1, GB).rearrange(
                                    "p g (h d) -> p g h d", h=H),
                                op=mybir.AluOpType.mult)
        # binary-tree reduce over D=64 within each head
        tprev = wm
        half = D
        while half > 1:
            half //= 2
            tn = pool.tile([P, GB, H, half], bf, tag=f"tr{half}")
            nc.vector.tensor_tensor(out=tn[:], in0=tprev[:, :, :, 0:half],
                                    in1=tprev[:, :, :, half:2 * half],
                                    op=mybir.AluOpType.add)
            tprev = tn
        p_all = pool.tile([P, GB, H], bf, tag="p")
        nc.scalar.activation(p_all[:], tprev[:, :, :, 0], Act.Exp)

        denre = psD.tile([P, GB, 2 * H], f32, tag="denre")
        den_t = denre[0:W, :, 0:H]
        re_ps = denre[:, :, H:2 * H]
        for j in range(GB):
            nc.tensor.matmul(den_t[:, j, :],
                             lhsT=i0b[:, j, :], rhs=p_all[:, j, :],
                             start=True, stop=True)
        dens = pool.tile([W, GB, H], f32, tag="dens")
        nc.vector.tensor_scalar_add(dens[:], den_t[:], 1e-16)
        recip = pool.tile([W, GB, H], bf, tag="recip")
        with nc.allow_low_precision(reason="attn denom O(1)"):
            nc.vector.reciprocal(recip[:], dens[:])
        for j in range(GB):
            nc.tensor.matmul(re_ps[:, j, :], lhsT=i0tb[:, j, :],
                             rhs=recip[:, j, :],
                             start=True, stop=True)
        alpha = pool.tile([P, GB, H], bf, tag="alpha")
        nc.vector.tensor_tensor(out=alpha[:], in0=p_all[:], in1=re_ps[:],
                                op=mybir.AluOpType.mult)
        # fold alpha into indicator: i0a[p,g,h,w] = i0[p,g,w]*alpha[p,g,h]
        i0a = pool.tile([P, GB, H, W], bf, tag="i0a")
        nc.vector.tensor_tensor(out=i0a[:], in0=_bc_mid(i0b[:], 2, H),
                                in1=alpha[:].to_broadcast([P, GB, H, W]),
                                op=mybir.AluOpType.mult)
        # transposed aggregation: hT[f, j, c, w] = sum_e alpha*xl, f=64h+d
        hT_ps = psN.tile([P, GB, 2, W], f32, tag="hT")
        for j in range(GB):
            for h in range(H):
                nc.tensor.matmul(
                    hT_ps[64 * (h % 2):64 * (h % 2) + 64, j, h // 2, :],
                    lhsT=xl_g[:, j, D * h:D * (h + 1)],
                    rhs=i0a[:, j, h, :],
                    start=True, stop=True)
        hbT = pool.tile([P, 2, GB * W], bf, tag="hbT")
        for c in range(2):
            nsbT = pool.tile([P, GB, W], bf, tag=f"nsbT{c}",
                             name=f"nsbT{c}")
            nc.scalar.activation(nsbT[:], hT_ps[:, :, c, :], Act.Copy)
            nc.vector.tensor_tensor(out=hbT[:, c, :], in0=nsbT[:].rearrange(
                                        "p j w -> p (j w)"),
                                    in1=biasT[:, c, :],
                                    op=mybir.AluOpType.add)

        if layer == 1:
            # silu(x) = x / (1 + exp(-x)), in transposed layout
            hT_sb = pool.tile([P, 2, GB * W], bf, tag="hTs")
            for c in range(2):
                sg = pool.tile([P, GB * W], bf, tag=f"sg{c}")
                nc.scalar.activation(sg[:], hbT[:, c, :], Act.Exp,
                                     scale=-1.0)
                d2 = pool.tile([P, GB * W], f32, tag=f"d2{c}")
                nc.vector.tensor_scalar_add(d2[:], sg[:], 1.0)
                rc = pool.tile([P, GB * W], bf, tag=f"rc{c}")
                with nc.allow_low_precision(reason="silu denom in [1,2]"):
                    nc.vector.reciprocal(rc[:], d2[:])
                nc.vector.tensor_tensor(out=hT_sb[:, c, :],
                                        in0=hbT[:, c, :], in1=rc[:],
                                        op=mybir.AluOpType.mult)
            # fused layer-2 transforms: hT_sb chunks ARE the lhsT
            lhs = [hT_sb[:, 0, :], hT_sb[:, 1, :]]
            _transform_chunk(nc, psN, pool, lhs, consts["wl2_sb"], 2,
                             consts["bl2r"],
                             xl2_slot[b * P:(b + 1) * P, :], "xl2")
            _transform_chunk(nc, psN, pool, lhs, consts["wr2_sb"], 2,
                             consts["br2r"],
                             xr2_slot[b * P:(b + 1) * P, :], "xr2")
        else:
            # stage h2^T to DRAM, read back transposed, pool immediately
            for c in range(2):
                nc.sync.dma_start(
                    h2T_dram[c * P:(c + 1) * P, b * P:(b + 1) * P],
                    hbT[:, c, :])
                h2c = pool.tile([P, P], bf, tag=f"h2c{c}", name=f"h2c{c}")
                nc.sync.dma_start(
                    h2c[:],
                    h2T_dram[c * P:(c + 1) * P, b * P:(b + 1) * P],
                    transpose=True)
                nc.tensor.matmul(pool_ps[c][:],
                                 lhsT=gind_b[:], rhs=h2c[:],
                                 start=(b == 0), stop=(b == NB - 1))


def build_program(T):
    nc = bass.Bass()
    S = T * W
    NB = T // GB
    d_in = {}
    for name, shape, dt in [
        ("xT", [IN, S], bf),
        ("wl1", [IN, HD], bf), ("wr1", [IN, HD], bf),
        ("wl2", [HD, HD], bf), ("wr2", [HD, HD], bf),
        ("bl1r", [P, HD], bf), ("br1r", [P, HD], bf),
        ("bl2r", [P, HD], bf), ("br2r", [P, HD], bf),
        ("bias1T", [P, 2 * P], bf), ("bias2T", [P, 2 * P], bf),
        ("att1r", [P, HD], bf), ("att2r", [P, HD], bf),
        ("srcidx", [P, T], i32),
        ("i0", [P, T * W], bf), ("i0t", [W, T * P], bf),
        ("gind", [P, NB * G], bf),
    ]:
        d_in[name] = nc.declare_dram_parameter(name, shape, dt,
                                               isOutput=False)
    out = nc.declare_dram_parameter("out", [G, HD], f32, isOutput=True)
    dbg = os.environ.get("GAT_DEBUG", "0") == "1"
    if dbg:
        dbg_xl1 = nc.declare_dram_parameter("dbg_xl1", [S, HD], bf,
                                            isOutput=True)
        dbg_xl2 = nc.declare_dram_parameter("dbg_xl2", [S, HD], bf,
                                            isOutput=True)
        dbg_pool = nc.declare_dram_parameter("dbg_pool", [G, HD], f32,
                                             isOutput=True)
        dbg_h2T = nc.declare_dram_parameter("dbg_h2T", [2 * P, S], bf,
                                            isOutput=True)

    xl1_slot = nc.dram_tensor("xl1_slot", [S, HD], bf)
    xr1_slot = nc.dram_tensor("xr1_slot", [S, HD], bf)
    xl1_full = nc.dram_tensor("xl1_full", [NCORES * S, HD], bf,
                              addr_space="Shared")
    xl2_slot = nc.dram_tensor("xl2_slot", [S, HD], bf)
    xr2_slot = nc.dram_tensor("xr2_slot", [S, HD], bf)
    xl2_full = nc.dram_tensor("xl2_full", [NCORES * S, HD], bf,
                              addr_space="Shared")
    h2T_dram = nc.dram_tensor("h2T_dram", [2 * P, S], bf)
    pool_loc = nc.dram_tensor("pool_loc", [G, HD], f32)
    pool_sum = nc.dram_tensor("pool_sum", [G, HD], f32, addr_space="Shared")

    with tile.TileContext(nc) as tc, ExitStack() as ctx:
        from concourse.masks import make_identity
        gc = ctx.enter_context(tc.tile_pool(name="gc", bufs=1))
        ident = gc.tile([P, P], f32)
        make_identity(nc, ident[:])
        ident_bf = gc.tile([P, P], bf)
        nc.vector.tensor_copy(ident_bf[:], ident[:])

        consts = {}
        for nm in ("att1r", "att2r", "bl1r", "br1r", "bl2r", "br2r"):
            t = gc.tile([P, HD], bf, tag=nm, name=f"c_{nm}")
            nc.sync.dma_start(t[:], d_in[nm][:, :])
            consts[nm] = t
        for nm in ("bias1T", "bias2T"):
            t = gc.tile([P, 2, P], bf, tag=nm, name=f"c_{nm}")
            nc.sync.dma_start(t[:].rearrange("p c k -> p (c k)"),
                              d_in[nm][:, :])
            consts[nm] = t
        for nm, src in (("wl2_sb", "wl2"), ("wr2_sb", "wr2")):
            t = gc.tile([P, 2, HD], bf, tag=nm, name=f"c_{nm}")
            for kk in range(2):
                nc.sync.dma_start(t[:, kk, :],
                                  d_in[src][kk * P:(kk + 1) * P, :])
            consts[nm] = t

        # phase A: layer-1 transforms over slot chunks
        with ExitStack() as c1:
            tp = c1.enter_context(tc.tile_pool(name="tfA", bufs=2))
            tps = c1.enter_context(tc.tile_pool(name="tfA_ps", bufs=2,
                                                space="PSUM"))
            cw = c1.enter_context(tc.tile_pool(name="tfA_c", bufs=1))
            xT_sb = cw.tile([IN, S], bf)
            nc.sync.dma_start(xT_sb[:], d_in["xT"][:, :])
            wl1_sb = cw.tile([IN, 1, HD], bf)
            nc.sync.dma_start(wl1_sb[:, 0, :], d_in["wl1"][:, :])
            wr1_sb = cw.tile([IN, 1, HD], bf)
            nc.sync.dma_start(wr1_sb[:, 0, :], d_in["wr1"][:, :])
            for c in range(NB):
                lhs = [xT_sb[:, c * P:(c + 1) * P]]
                _transform_chunk(nc, tps, tp, lhs, wl1_sb, 1,
                                 consts["bl1r"],
                                 xl1_slot[c * P:(c + 1) * P, :], "xl1")
                _transform_chunk(nc, tps, tp, lhs, wr1_sb, 1,
                                 consts["br1r"],
                                 xr1_slot[c * P:(c + 1) * P, :], "xr1")

        # phase B: AllGather xl1
        nc.gpsimd.collective_compute(
            "AllGather", mybir.AluOpType.bypass,
            replica_groups=[list(range(NCORES))],
            ins=[xl1_slot[0:S, :]], outs=[xl1_full[:, :]])

        # phase C: edge layer 1 (+ fused layer-2 transforms)
        with ExitStack() as c2:
            _edge_phase(nc, tc, c2, T, 1, xl1_full, xr1_slot, d_in,
                        ident_bf, consts, xl2_slot=xl2_slot,
                        xr2_slot=xr2_slot)

        # phase D: AllGather xl2
        nc.gpsimd.collective_compute(
            "AllGather", mybir.AluOpType.bypass,
            replica_groups=[list(range(NCORES))],
            ins=[xl2_slot[0:S, :]], outs=[xl2_full[:, :]])

        # phase E: edge layer 2 + pool accumulation
        with ExitStack() as c3:
            plp = c3.enter_context(tc.tile_pool(name="poolps", bufs=1,
                                                space="PSUM"))
            pool_ps = [plp.tile([G, P], f32, tag=f"pps{c}", name=f"pps{c}")
                       for c in range(2)]
            _edge_phase(nc, tc, c3, T, 2, xl2_full, xr2_slot, d_in,
                        ident_bf, consts, pool_ps=pool_ps,
                        h2T_dram=h2T_dram)
            psb = c3.enter_context(tc.tile_pool(name="poolsb", bufs=1))
            pool_sb = psb.tile([G, HD], f32)
            for c in range(2):
                nc.scalar.activation(pool_sb[:, P * c:P * (c + 1)],
                                     pool_ps[c][:],
                                     mybir.ActivationFunctionType.Copy)
            nc.sync.dma_start(pool_loc[:, :], pool_sb[:])
            if dbg:
                nc.sync.dma_start(dbg_pool[:, :], pool_sb[:])
                dpool = c3.enter_context(tc.tile_pool(name="dbgp", bufs=2))
                for c in range(NB):
                    dt1 = dpool.tile([P, HD], bf, tag="d1")
                    nc.sync.dma_start(dt1[:], xl1_slot[c * P:(c + 1) * P, :])
                    nc.sync.dma_start(dbg_xl1[c * P:(c + 1) * P, :], dt1[:])
                    dt2 = dpool.tile([P, HD], bf, tag="d2")
                    nc.sync.dma_start(dt2[:], xl2_slot[c * P:(c + 1) * P, :])
                    nc.sync.dma_start(dbg_xl2[c * P:(c + 1) * P, :], dt2[:])
                    for cc in range(2):
                        dt3 = dpool.tile([P, P], bf, tag="d3", name="dt3")
                        nc.sync.dma_start(
                            dt3[:],
                            h2T_dram[cc * P:(cc + 1) * P,
                                     c * P:(c + 1) * P])
                        nc.sync.dma_start(
                            dbg_h2T[cc * P:(cc + 1) * P,
                                    c * P:(c + 1) * P], dt3[:])
            nc.gpsimd.collective_compute(
                "AllReduce", mybir.AluOpType.add,
                replica_groups=[list(range(NCORES))],
                ins=[pool_loc[:, :]], outs=[pool_sum[:, :]])
            outt = psb.tile([G, HD], f32)
            nc.sync.dma_start(outt[:], pool_sum[:, :])
            nc.sync.dma_start(out[:, :], outt[:])

    return nc


# ------------------------------------------------------------------- driver


def _pjrt_prepare(nc, in_maps):
    """Build the jitted 8-core executable + device-resident inputs."""
    import jax
    from jax.sharding import Mesh, PartitionSpec
    from jax.experimental.shard_map import shard_map
    from concourse import bass2jax

    bass2jax.install_neuronx_cc_hook()
    n_cores = len(in_maps)
    partition_name = (nc.partition_id_tensor.name
                      if nc.partition_id_tensor else None)
    in_names, out_names, out_avals, zero_outs = [], [], [], []
    for alloc in nc.m.functions[0].allocations:
        if not isinstance(alloc, mybir.MemoryLocationSet):
            continue
        name = alloc.memorylocations[0].name
        if alloc.kind == "ExternalInput":
            if name != partition_name:
                in_names.append(name)
        elif alloc.kind == "ExternalOutput":
            out_names.append(name)
            shape = tuple(alloc.tensor_shape)
            dtype = mybir.dt.np(alloc.dtype)
            out_avals.append(jax.core.ShapedArray(shape, dtype))
            zero_outs.append(np.zeros(shape, dtype))
    n_params = len(in_names)
    n_outs = len(out_avals)
    all_in_names = list(in_names) + list(out_names)
    if partition_name is not None:
        all_in_names.append(partition_name)

    def _body(*args):
        operands = list(args)
        if partition_name is not None:
            operands.append(bass2jax.partition_id_tensor())
        outs = bass2jax._bass_exec_p.bind(
            *operands,
            out_avals=tuple(out_avals),
            in_names=tuple(all_in_names),
            out_names=tuple(out_names),
            lowering_input_output_aliases=(),
            sim_require_finite=True,
            sim_require_nnan=True,
            nc=nc,
        )
        return tuple(outs)

    devices = jax.devices()[:n_cores]
    mesh = Mesh(np.asarray(devices), ("core",))
    in_specs = (PartitionSpec("core"),) * (n_params + n_outs)
    out_specs = (PartitionSpec("core"),) * len(out_names)
    sharded = jax.jit(
        shard_map(_body, mesh=mesh, in_specs=in_specs, out_specs=out_specs,
                  check_rep=False),
        keep_unused=True)
    concat_in = [
        np.concatenate([np.asarray(in_maps[c][nm]) for c in range(n_cores)],
                       axis=0)
        for nm in in_names
    ]
    from jax.sharding import NamedSharding
    sh = NamedSharding(mesh, PartitionSpec("core"))
    dev_in = [jax.device_put(a, sh) for a in concat_in]

    dev_zeros = [jax.device_put(
        np.zeros((n_cores * z.shape[0], *z.shape[1:]), z.dtype), sh)
        for z in zero_outs]

    def run_fn():
        outs = sharded(*dev_in, *dev_zeros)
        import jax as _j
        _j.block_until_ready(outs)
        return outs

    return run_fn, out_names, out_avals


def kernel(**inputs):
    in_maps, T = _host_prep(inputs)
    nc = build_program(T)
    _legalize_waits(nc)
    run_fn, out_names, out_avals = _pjrt_prepare(nc, in_maps)
    outs = run_fn()   # compile + first exec
    if os.environ.get("GAT_BENCH", "0") == "1":
        import time
        times = []
        for _ in range(int(os.environ.get("GAT_BENCH_ITERS", "5"))):
            t0 = time.perf_counter()
            outs = run_fn()
            times.append(time.perf_counter() - t0)
        kernel.last_exec_time_ns = int(min(times) * 1e9)
        kernel.bench_times = times
    i = out_names.index("out")
    full = np.asarray(outs[i]).reshape(NCORES, *out_avals[i].shape)
    return np.asarray(full[0], np.float32)


# revision 45
# speedup vs baseline: 1.3110x; 1.1076x over previous
"""GATv2 x2 + global mean pool on 8 Trainium2 NeuronCores (Bass/Tile), v2.

Slot-space layout (dst-sharded, uniform tiles):
  - Edges sorted by dst; per core, greedy tiles of <=8 dst nodes ("window
    slots") and <=128 edges. Tile t owns slot rows [8t, 8t+8); a batch of
    16 tiles = 128 consecutive slot rows, so ALL window-side accesses
    (xr rows, pool indicators) are plain contiguous DMAs.
  - Node tables (xl/xr) are computed in slot space; xl tables are
    AllGathered to a [8*S, 256] global table; per-edge xl rows come via
    per-tile indirect DMA gathers with global-slot indices.
  - Scores: tables have no aux cols; e = sum_hd att*leaky(S) computed as
    Prelu(S) on ACT, att-mul + binary-tree head reduction on DVE.
  - Aggregation: alpha is folded into the one-hot indicator (i0a = i0 *
    alpha per head), so num = sum per head via 4 matmuls per tile and the
    per-edge [128,256] alpha-broadcast multiply disappears.
  - silu via exp table (1/(1+e^-x)*x) to keep a single ACT table set
    (prelu/exp/copy) with zero table reloads.
  - Layer-2 transforms are fused into the layer-1 edge loop: h tiles are
    PE-transposed in SBUF and immediately transformed; h never goes to
    DRAM.
  - Global mean pool via per-batch [128,64] indicator matmul accumulated
    in PSUM across all batches; final AllReduce over [64,256].
"""
import sys

sys.path.insert(0, "/opt/trn_rl_repo")
sys.path.insert(0, "/opt/pypackages")

import os
from contextlib import ExitStack

import numpy as np
import ml_dtypes

import concourse.bass as bass
import concourse.mybir as mybir
import concourse.tile as tile

BF16 = ml_dtypes.bfloat16
bf = mybir.dt.bfloat16
f32 = mybir.dt.float32
i32 = mybir.dt.int32

N, E, G = 50000, 800000, 64
IN, H, D = 128, 4, 64
HD = H * D
NCORES = 8
NPC = N // NCORES
P = 128                      # edge slots per tile
W = 8                        # window (dst-node) slots per tile
GB = 16                      # tiles per batch (= 128 slot rows)
NEG = 0.2

# ---------------------------------------------------------------- host prep


def _tile_core(dst_l):
    """Bin-pack local nodes into tiles of <=W nodes and <=P edges.

    Nodes are relabeled freely within the core (everything downstream is
    slot-indexed). Snake-deal by degree, then repair overloaded bins.
    Returns (tiles, counts): tiles = list of node-id arrays.
    """
    counts = np.bincount(dst_l, minlength=NPC)
    assert counts.max() <= P, counts.max()
    order = np.argsort(-counts, kind="stable")
    import heapq
    nb = max((NPC + W - 1) // W, int(np.ceil(counts.sum() / P)))
    while True:
        bins = [[] for _ in range(nb)]
        load = np.zeros(nb, np.int64)
        heap = [(0, b) for b in range(nb)]
        heapq.heapify(heap)
        ok = True
        for n in order:
            c = int(counts[n])
            parked = []
            placed = False
            while heap:
                ld, b = heapq.heappop(heap)
                if ld + c <= P and len(bins[b]) < W:
                    bins[b].append(n)
                    load[b] = ld + c
                    if len(bins[b]) < W:
                        heapq.heappush(heap, (ld + c, b))
                    placed = True
                    break
                parked.append((ld, b))
                if ld + c > P:
                    break
            for it in parked:
                heapq.heappush(heap, it)
            if not placed:
                ok = False
                break
        if ok:
            return [np.sort(np.asarray(b, np.int64)) for b in bins], counts
        nb += 4


def _host_prep(inputs):
    x = np.asarray(inputs["x"], np.float32)
    ei = np.asarray(inputs["edge_index"]).astype(np.int64)
    batch = np.asarray(inputs["batch"]).astype(np.int64)

    src, dst = ei[0], ei[1]
    order = np.argsort(dst, kind="stable")
    src_s, dst_s = src[order].astype(np.int64), dst[order].astype(np.int64)
    core_of = dst_s // NPC

    cnt = np.bincount(batch, minlength=G).astype(np.float32)
    inv_cnt = (1.0 / np.maximum(cnt, 1.0)).astype(np.float32)

    per_core = []
    tiles_pc = []
    for k in range(NCORES):
        m = core_of == k
        s_k, d_k = src_s[m], dst_s[m] - k * NPC
        tiles, counts = _tile_core(d_k)
        per_core.append((s_k, d_k, counts))
        tiles_pc.append(tiles)

    T = max(len(t) for t in tiles_pc)
    T = ((T + GB - 1) // GB) * GB
    S = T * W
    NB = T // GB

    # global slot ids
    slot_of = np.full(N, -1, np.int64)
    for k in range(NCORES):
        for t, nl in enumerate(tiles_pc[k]):
            slot_of[nl + k * NPC] = k * S + t * W + np.arange(len(nl))
    assert (slot_of >= 0).all()

    w_bf = {}
    for nm in ("Wl1", "Wr1", "Wl2", "Wr2"):
        w_bf[nm] = np.asarray(inputs[nm], np.float32).astype(BF16)
    rep = lambda v: np.repeat(np.asarray(v, np.float32)[None, :], P, 0).astype(BF16)
    repT = lambda v: np.broadcast_to(
        np.asarray(v, np.float32).reshape(2, P).transpose(1, 0)[:, :, None],
        (P, 2, P)).astype(BF16).copy()
    att1r = rep(np.asarray(inputs["att1"], np.float32).reshape(HD))
    att2r = rep(np.asarray(inputs["att2"], np.float32).reshape(HD))
    bl1r, br1r = rep(inputs["bl1"]), rep(inputs["br1"])
    bl2r, br2r = rep(inputs["bl2"]), rep(inputs["br2"])
    bias1T, bias2T = repT(inputs["bias1"]), repT(inputs["bias2"])

    in_maps = []
    for k in range(NCORES):
        s_k, d_k, counts = per_core[k]
        tiles = tiles_pc[k]
        starts = np.zeros(NPC + 1, np.int64)
        np.cumsum(counts, out=starts[1:])

        xT = np.zeros((IN, S), np.float32)
        srcidx = np.zeros((P, T), np.int32)
        i0 = np.zeros((P, T, W), np.float32)
        i0t = np.zeros((W, T, P), np.float32)
        gind = np.zeros((P, NB, G), np.float32)
        for t, nl in enumerate(tiles):
            nw = len(nl)
            nodes = nl + k * NPC
            xT[:, t * W:t * W + nw] = x[nodes].T
            gslot = t * W + np.arange(nw)          # local slot of window rows
            brow, prow = divmod(gslot, P)          # batch id / row within
            gind[prow, brow, batch[nodes]] = inv_cnt[batch[nodes]]
            tile_cnt = counts[nl]
            srcs = np.concatenate(
                [s_k[starts[n]:starts[n + 1]] for n in nl]) if nw else                 np.zeros(0, np.int64)
            ke = int(tile_cnt.sum())
            if ke:
                srcidx[:ke, t] = slot_of[srcs]
                offs = np.repeat(np.arange(nw), tile_cnt)
                i0[np.arange(ke), t, offs] = 1.0
                i0t[offs, t, np.arange(ke)] = 1.0

        in_maps.append({
            "xT": xT.astype(BF16),
            "wl1": w_bf["Wl1"], "wr1": w_bf["Wr1"],
            "wl2": w_bf["Wl2"], "wr2": w_bf["Wr2"],
            "bl1r": bl1r, "br1r": br1r, "bl2r": bl2r, "br2r": br2r,
            "bias1T": bias1T.reshape(P, 2 * P), "bias2T": bias2T.reshape(P, 2 * P),
            "att1r": att1r, "att2r": att2r,
            "srcidx": srcidx,
            "i0": i0.reshape(P, T * W).astype(BF16),
            "i0t": i0t.reshape(W, T * P).astype(BF16),
            "gind": gind.reshape(P, NB * G).astype(BF16),
        })
    return in_maps, T

# ------------------------------------------------------------- bass program


def _legalize_waits(nc):
    """walrus allows 1 sync wait on DMA/CTRL instrs, 2 on compute instrs.
    Hoist excess waits onto same-engine NoOps inserted just before."""
    n_ins = 0
    for blk in nc.m.functions[0].blocks:
        out = []
        for inst in blk.instructions:
            si = inst.sync_info
            waits = list(si.on_wait) if (si is not None and si.on_wait) else []
            lim = 1
            if len(waits) > lim:
                for wchunk in waits[:-lim]:
                    nop = mybir.InstNoOp(name=f"waitnop_{n_ins}_{inst.name}",
                                         ins=[], outs=[])
                    nop.engine = inst.engine
                    nop.sync_info = mybir.SyncInfo(on_wait=[wchunk],
                                                   on_update=[])
                    out.append(nop)
                    n_ins += 1
                si.on_wait = waits[-lim:]
            out.append(inst)
        blk.instructions = out
    return n_ins


def _bc_mid(ap, axis, n):
    """Insert a stride-0 broadcast dim of size n at `axis` into an AP."""
    lst = [list(d) for d in ap.ap]
    lst.insert(axis, [0, n])
    return bass.AP(ap.tensor, ap.offset, lst)


def _transform_chunk(nc, psum, pool, lhs_chunks, w_sb, nk, brep, dst_rows,
                     tag):
    """dst_rows <- (lhsT.T @ w) + bias_rep, bf16."""
    ps = psum.tile([P, HD], f32, tag="ps_tf", name=f"ps_{tag}")
    for kk in range(nk):
        nc.tensor.matmul(ps[:], lhsT=lhs_chunks[kk], rhs=w_sb[:, kk, :],
                         start=(kk == 0), stop=(kk == nk - 1))
    o0 = pool.tile([P, HD], bf, tag=f"o0_{tag}")
    nc.scalar.activation(o0[:], ps[:], mybir.ActivationFunctionType.Copy)
    o1 = pool.tile([P, HD], bf, tag=f"o1_{tag}")
    nc.vector.tensor_tensor(out=o1[:], in0=o0[:], in1=brep[:],
                            op=mybir.AluOpType.add)
    nc.sync.dma_start(dst_rows, o1[:])


def _edge_phase(nc, tc, ctx, T, layer, xl_full, xr_slot, d_in, ident_bf,
                consts, xl2_slot=None, xr2_slot=None, pool_ps=None,
                h2T_dram=None):
    NB = T // GB
    S_rows = T * W
    pool = ctx.enter_context(tc.tile_pool(name=f"e{layer}_sb", bufs=2))
    gpool = ctx.enter_context(tc.tile_pool(name=f"e{layer}_g", bufs=4))
    psS = ctx.enter_context(tc.tile_pool(name=f"e{layer}_psS", bufs=2,
                                         space="PSUM"))
    psN = ctx.enter_context(tc.tile_pool(name=f"e{layer}_psN", bufs=2,
                                         space="PSUM"))
    psD = psN
    att_rep = consts["att1r" if layer == 1 else "att2r"]
    biasT = consts["bias1T" if layer == 1 else "bias2T"]

    Act = mybir.ActivationFunctionType
    for b in range(NB):
        t0 = b * GB
        sidx = gpool.tile([P, GB], i32, tag="sidx")
        nc.sync.dma_start(sidx[:], d_in["srcidx"][:, t0:t0 + GB])
        i0b = gpool.tile([P, GB, W], bf, tag="i0b")
        nc.sync.dma_start(i0b[:], d_in["i0"][:, t0 * W:(t0 + GB) * W])
        i0tb = gpool.tile([W, GB, P], bf, tag="i0tb")
        nc.sync.dma_start(i0tb[:], d_in["i0t"][:, t0 * P:(t0 + GB) * P])
        # window rows in window-major layout: partition w, free (j, c)
        xr_t = gpool.tile([W, GB, HD], bf, tag="xr")
        nc.sync.dma_start(
            xr_t[:],
            xr_slot[b * P:(b + 1) * P, :].rearrange("(j w) c -> w j c", w=W))
        if layer == 2:
            gind_b = gpool.tile([P, G], bf, tag="gind")
            nc.sync.dma_start(gind_b[:], d_in["gind"][:, b * G:(b + 1) * G])
        xl_g = gpool.tile([P, GB, HD], bf, tag="xlg")
        for j in range(GB):
            nc.gpsimd.indirect_dma_start(
                out=xl_g[:, j, :], out_offset=None, in_=xl_full[:, :],
                in_offset=bass.IndirectOffsetOnAxis(
                    ap=sidx[:, j:j + 1], axis=0))

        # scores: S = xr[dst] + xl[src]; m = leaky(S)
        m_all = pool.tile([P, GB, HD], bf, tag="m")
        for j in range(GB):
            S_ps = psS.tile([P, HD], f32, tag="S")
            nc.tensor.matmul(S_ps[:], lhsT=i0tb[:, j, :],
                             rhs=xr_t[:, j, :],
                             start=True, stop=False)
            nc.tensor.matmul(S_ps[:], lhsT=ident_bf[:], rhs=xl_g[:, j, :],
                             start=False, stop=True)
            nc.scalar.activation(m_all[:, j, :], S_ps[:], Act.Prelu,
                                 alpha=NEG)
        wm = pool.tile([P, GB, H, D], bf, tag="wm")
        nc.vector.tensor_tensor(out=wm[:],
                                in0=m_all[:].rearrange(
                                    "p g (h d) -> p g h d", h=H),
                                in1=_bc_mid(att_rep[:], 1, GB).rearrange(
                                    "p g (h d) -> p g h d", h=H),
                                op=mybir.AluOpType.mult)
        # binary-tree reduce over D=64 within each head
        tprev = wm
        half = D
        while half > 1:
            half //= 2
            tn = pool.tile([P, GB, H, half], bf, tag=f"tr{half}")
            nc.vector.tensor_tensor(out=tn[:], in0=tprev[:, :, :, 0:half],
                                    in1=tprev[:, :, :, half:2 * half],
                                    op=mybir.AluOpType.add)
            tprev = tn
        p_all = pool.tile([P, GB, H], bf, tag="p")
        nc.scalar.activation(p_all[:], tprev[:, :, :, 0], Act.Exp)

        denre = psD.tile([P, GB, 2 * H], f32, tag="denre")
        den_t = denre[0:W, :, 0:H]
        re_ps = denre[:, :, H:2 * H]
        for j in range(GB):
            nc.tensor.matmul(den_t[:, j, :],
                             lhsT=i0b[:, j, :], rhs=p_all[:, j, :],
                             start=True, stop=True)
        dens = pool.tile([W, GB, H], f32, tag="dens")
        nc.vector.tensor_scalar_add(dens[:], den_t[:], 1e-16)
        recip = pool.tile([W, GB, H], bf, tag="recip")
        with nc.allow_low_precision(reason="attn denom O(1)"):
            nc.vector.reciprocal(recip[:], dens[:])
        for j in range(GB):
            nc.tensor.matmul(re_ps[:, j, :], lhsT=i0tb[:, j, :],
                             rhs=recip[:, j, :],
                             start=True, stop=True)
        alpha = pool.tile([P, GB, H], bf, tag="alpha")
        nc.vector.tensor_tensor(out=alpha[:], in0=p_all[:], in1=re_ps[:],
                                op=mybir.AluOpType.mult)
        # fold alpha into indicator: i0a[p,g,h,w] = i0[p,g,w]*alpha[p,g,h]
        i0a = pool.tile([P, GB, H, W], bf, tag="i0a")
        nc.vector.tensor_tensor(out=i0a[:], in0=_bc_mid(i0b[:], 2, H),
                                in1=alpha[:].to_broadcast([P, GB, H, W]),
                                op=mybir.AluOpType.mult)
        # transposed aggregation: hT[f, j, c, w] = sum_e alpha*xl, f=64h+d
        hT_ps = psN.tile([P, GB, 2, W], f32, tag="hT")
        for j in range(GB):
            for h in range(H):
                nc.tensor.matmul(
                    hT_ps[64 * (h % 2):64 * (h % 2) + 64, j, h // 2, :],
                    lhsT=xl_g[:, j, D * h:D * (h + 1)],
                    rhs=i0a[:, j, h, :],
                    start=True, stop=True)
        hbT = pool.tile([P, 2, GB * W], bf, tag="hbT")
        for c in range(2):
            nsbT = pool.tile([P, GB, W], bf, tag=f"nsbT{c}",
                             name=f"nsbT{c}")
            nc.scalar.activation(nsbT[:], hT_ps[:, :, c, :], Act.Copy)
            nc.vector.tensor_tensor(out=hbT[:, c, :], in0=nsbT[:].rearrange(
                                        "p j w -> p (j w)"),
                                    in1=biasT[:, c, :],
                                    op=mybir.AluOpType.add)

        if layer == 1:
            # silu(x) = x / (1 + exp(-x)), in transposed layout
            hT_sb = pool.tile([P, 2, GB * W], bf, tag="hTs")
            for c in range(2):
                sg = pool.tile([P, GB * W], bf, tag=f"sg{c}")
                nc.scalar.activation(sg[:], hbT[:, c, :], Act.Exp,
                                     scale=-1.0)
                d2 = pool.tile([P, GB * W], f32, tag=f"d2{c}")
                nc.vector.tensor_scalar_add(d2[:], sg[:], 1.0)
                rc = pool.tile([P, GB * W], bf, tag=f"rc{c}")
                with nc.allow_low_precision(reason="silu denom in [1,2]"):
                    nc.vector.reciprocal(rc[:], d2[:])
                nc.vector.tensor_tensor(out=hT_sb[:, c, :],
                                        in0=hbT[:, c, :], in1=rc[:],
                                        op=mybir.AluOpType.mult)
            # fused layer-2 transforms: hT_sb chunks ARE the lhsT
            lhs = [hT_sb[:, 0, :], hT_sb[:, 1, :]]
            _transform_chunk(nc, psN, pool, lhs, consts["wl2_sb"], 2,
                             consts["bl2r"],
                             xl2_slot[b * P:(b + 1) * P, :], "xl2")
            _transform_chunk(nc, psN, pool, lhs, consts["wr2_sb"], 2,
                             consts["br2r"],
                             xr2_slot[b * P:(b + 1) * P, :], "xr2")
        else:
            # stage h2^T to DRAM, read back transposed, pool immediately
            for c in range(2):
                nc.sync.dma_start(
                    h2T_dram[c * P:(c + 1) * P, b * P:(b + 1) * P],
                    hbT[:, c, :])
                h2c = pool.tile([P, P], bf, tag=f"h2c{c}", name=f"h2c{c}")
                nc.sync.dma_start(
                    h2c[:],
                    h2T_dram[c * P:(c + 1) * P, b * P:(b + 1) * P],
                    transpose=True)
                nc.tensor.matmul(pool_ps[c][:],
                                 lhsT=gind_b[:], rhs=h2c[:],
                                 start=(b == 0), stop=(b == NB - 1))


def build_program(T):
    nc = bass.Bass()
    S = T * W
    NB = T // GB
    d_in = {}
    for name, shape, dt in [
        ("xT", [IN, S], bf),
        ("wl1", [IN, HD], bf), ("wr1", [IN, HD], bf),
        ("wl2", [HD, HD], bf), ("wr2", [HD, HD], bf),
        ("bl1r", [P, HD], bf), ("br1r", [P, HD], bf),
        ("bl2r", [P, HD], bf), ("br2r", [P, HD], bf),
        ("bias1T", [P, 2 * P], bf), ("bias2T", [P, 2 * P], bf),
        ("att1r", [P, HD], bf), ("att2r", [P, HD], bf),
        ("srcidx", [P, T], i32),
        ("i0", [P, T * W], bf), ("i0t", [W, T * P], bf),
        ("gind", [P, NB * G], bf),
    ]:
        d_in[name] = nc.declare_dram_parameter(name, shape, dt,
                                               isOutput=False)
    out = nc.declare_dram_parameter("out", [G, HD], f32, isOutput=True)
    dbg = os.environ.get("GAT_DEBUG", "0") == "1"
    if dbg:
        dbg_xl1 = nc.declare_dram_parameter("dbg_xl1", [S, HD], bf,
                                            isOutput=True)
        dbg_xl2 = nc.declare_dram_parameter("dbg_xl2", [S, HD], bf,
                                            isOutput=True)
        dbg_pool = nc.declare_dram_parameter("dbg_pool", [G, HD], f32,
                                             isOutput=True)
        dbg_h2T = nc.declare_dram_parameter("dbg_h2T", [2 * P, S], bf,
                                            isOutput=True)

    xl1_slot = nc.dram_tensor("xl1_slot", [S, HD], bf)
    xr1_slot = nc.dram_tensor("xr1_slot", [S, HD], bf)
    xl1_full = nc.dram_tensor("xl1_full", [NCORES * S, HD], bf,
                              addr_space="Shared")
    xl2_slot = nc.dram_tensor("xl2_slot", [S, HD], bf)
    xr2_slot = nc.dram_tensor("xr2_slot", [S, HD], bf)
    xl2_full = nc.dram_tensor("xl2_full", [NCORES * S, HD], bf,
                              addr_space="Shared")
    h2T_dram = nc.dram_tensor("h2T_dram", [2 * P, S], bf)
    pool_loc = nc.dram_tensor("pool_loc", [G, HD], f32)
    pool_sum = nc.dram_tensor("pool_sum", [G, HD], f32, addr_space="Shared")

    with tile.TileContext(nc) as tc, ExitStack() as ctx:
        from concourse.masks import make_identity
        gc = ctx.enter_context(tc.tile_pool(name="gc", bufs=1))
        ident = gc.tile([P, P], f32)
        make_identity(nc, ident[:])
        ident_bf = gc.tile([P, P], bf)
        nc.vector.tensor_copy(ident_bf[:], ident[:])

        consts = {}
        for nm in ("att1r", "att2r", "bl1r", "br1r", "bl2r", "br2r"):
            t = gc.tile([P, HD], bf, tag=nm, name=f"c_{nm}")
            nc.sync.dma_start(t[:], d_in[nm][:, :])
            consts[nm] = t
        for nm in ("bias1T", "bias2T"):
            t = gc.tile([P, 2, P], bf, tag=nm, name=f"c_{nm}")
            nc.sync.dma_start(t[:].rearrange("p c k -> p (c k)"),
                              d_in[nm][:, :])
            consts[nm] = t
        for nm, src in (("wl2_sb", "wl2"), ("wr2_sb", "wr2")):
            t = gc.tile([P, 2, HD], bf, tag=nm, name=f"c_{nm}")
            for kk in range(2):
                nc.sync.dma_start(t[:, kk, :],
                                  d_in[src][kk * P:(kk + 1) * P, :])
            consts[nm] = t

        # phase A: layer-1 transforms over slot chunks
        with ExitStack() as c1:
            tp = c1.enter_context(tc.tile_pool(name="tfA", bufs=2))
            tps = c1.enter_context(tc.tile_pool(name="tfA_ps", bufs=2,
                                                space="PSUM"))
            cw = c1.enter_context(tc.tile_pool(name="tfA_c", bufs=1))
            xT_sb = cw.tile([IN, S], bf)
            nc.sync.dma_start(xT_sb[:], d_in["xT"][:, :])
            wl1_sb = cw.tile([IN, 1, HD], bf)
            nc.sync.dma_start(wl1_sb[:, 0, :], d_in["wl1"][:, :])
            wr1_sb = cw.tile([IN, 1, HD], bf)
            nc.sync.dma_start(wr1_sb[:, 0, :], d_in["wr1"][:, :])
            for c in range(NB):
                lhs = [xT_sb[:, c * P:(c + 1) * P]]
                _transform_chunk(nc, tps, tp, lhs, wl1_sb, 1,
                                 consts["bl1r"],
                                 xl1_slot[c * P:(c + 1) * P, :], "xl1")
                _transform_chunk(nc, tps, tp, lhs, wr1_sb, 1,
                                 consts["br1r"],
                                 xr1_slot[c * P:(c + 1) * P, :], "xr1")

        # phase B: AllGather xl1
        nc.gpsimd.collective_compute(
            "AllGather", mybir.AluOpType.bypass,
            replica_groups=[list(range(NCORES))],
            ins=[xl1_slot[0:S, :]], outs=[xl1_full[:, :]])

        # phase C: edge layer 1 (+ fused layer-2 transforms)
        with ExitStack() as c2:
            _edge_phase(nc, tc, c2, T, 1, xl1_full, xr1_slot, d_in,
                        ident_bf, consts, xl2_slot=xl2_slot,
                        xr2_slot=xr2_slot)

        # phase D: AllGather xl2
        nc.gpsimd.collective_compute(
            "AllGather", mybir.AluOpType.bypass,
            replica_groups=[list(range(NCORES))],
            ins=[xl2_slot[0:S, :]], outs=[xl2_full[:, :]])

        # phase E: edge layer 2 + pool accumulation
        with ExitStack() as c3:
            plp = c3.enter_context(tc.tile_pool(name="poolps", bufs=1,
                                                space="PSUM"))
            pool_ps = [plp.tile([G, P], f32, tag=f"pps{c}", name=f"pps{c}")
                       for c in range(2)]
            _edge_phase(nc, tc, c3, T, 2, xl2_full, xr2_slot, d_in,
                        ident_bf, consts, pool_ps=pool_ps,
                        h2T_dram=h2T_dram)
            psb = c3.enter_context(tc.tile_pool(name="poolsb", bufs=1))
            pool_sb = psb.tile([G, HD], f32)
            for c in range(2):
                nc.scalar.activation(pool_sb[:, P * c:P * (c + 1)],
                                     pool_ps[c][:],
                                     mybir.ActivationFunctionType.Copy)
            nc.sync.dma_start(pool_loc[:, :], pool_sb[:])
            if dbg:
                nc.sync.dma_start(dbg_pool[:, :], pool_sb[:])
                dpool = c3.enter_context(tc.tile_pool(name="dbgp", bufs=2))
                for c in range(NB):
                    dt1 = dpool.tile([P, HD], bf, tag="d1")
                    nc.sync.dma_start(dt1[:], xl1_slot[c * P:(c + 1) * P, :])
                    nc.sync.dma_start(dbg_xl1[c * P:(c + 1) * P, :], dt1[:])
                    dt2 = dpool.tile([P, HD], bf, tag="d2")
                    nc.sync.dma_start(dt2[:], xl2_slot[c * P:(c + 1) * P, :])
                    nc.sync.dma_start(dbg_xl2[c * P:(c + 1) * P, :], dt2[:])
                    for cc in range(2):
                        dt3 = dpool.tile([P, P], bf, tag="d3", name="dt3")
                        nc.sync.dma_start(
                            dt3[:],
                            h2T_dram[cc * P:(cc + 1) * P,
                                     c * P:(c + 1) * P])
                        nc.sync.dma_start(
                            dbg_h2T[cc * P:(cc + 1) * P,
                                    c * P:(c + 1) * P], dt3[:])
            nc.gpsimd.collective_compute(
                "AllReduce", mybir.AluOpType.add,
                replica_groups=[list(range(NCORES))],
                ins=[pool_loc[:, :]], outs=[pool_sum[:, :]])
            outt = psb.tile([G, HD], f32)
            nc.sync.dma_start(outt[:], pool_sum[:, :])
            nc.sync.dma_start(out[:, :], outt[:])

    return nc


# ------------------------------------------------------------------- driver


def _pjrt_prepare(nc, in_maps):
    """Build the jitted 8-core executable + device-resident inputs."""
    import jax
    from jax.sharding import Mesh, PartitionSpec
    from jax.experimental.shard_map import shard_map
    from concourse import bass2jax

    bass2jax.install_neuronx_cc_hook()
    n_cores = len(in_maps)
    partition_name = (nc.partition_id_tensor.name
                      if nc.partition_id_tensor else None)
    in_names, out_names, out_avals, zero_outs = [], [], [], []
    for alloc in nc.m.functions[0].allocations:
        if not isinstance(alloc, mybir.MemoryLocationSet):
            continue
        name = alloc.memorylocations[0].name
        if alloc.kind == "ExternalInput":
            if name != partition_name:
                in_names.append(name)
        elif alloc.kind == "ExternalOutput":
            out_names.append(name)
            shape = tuple(alloc.tensor_shape)
            dtype = mybir.dt.np(alloc.dtype)
            out_avals.append(jax.core.ShapedArray(shape, dtype))
            zero_outs.append(np.zeros(shape, dtype))
    n_params = len(in_names)
    n_outs = len(out_avals)
    all_in_names = list(in_names) + list(out_names)
    if partition_name is not None:
        all_in_names.append(partition_name)

    def _body(*args):
        operands = list(args)
        if partition_name is not None:
            operands.append(bass2jax.partition_id_tensor())
        outs = bass2jax._bass_exec_p.bind(
            *operands,
            out_avals=tuple(out_avals),
            in_names=tuple(all_in_names),
            out_names=tuple(out_names),
            lowering_input_output_aliases=(),
            sim_require_finite=True,
            sim_require_nnan=True,
            nc=nc,
        )
        return tuple(outs)

    devices = jax.devices()[:n_cores]
    mesh = Mesh(np.asarray(devices), ("core",))
    in_specs = (PartitionSpec("core"),) * (n_params + n_outs)
    out_specs = (PartitionSpec("core"),) * len(out_names)
    sharded = jax.jit(
        shard_map(_body, mesh=mesh, in_specs=in_specs, out_specs=out_specs,
                  check_rep=False),
        keep_unused=True)
    concat_in = [
        np.concatenate([np.asarray(in_maps[c][nm]) for c in range(n_cores)],
                       axis=0)
        for nm in in_names
    ]
    from jax.sharding import NamedSharding
    sh = NamedSharding(mesh, PartitionSpec("core"))
    dev_in = [jax.device_put(a, sh) for a in concat_in]

    dev_zeros = [jax.device_put(
        np.zeros((n_cores * z.shape[0], *z.shape[1:]), z.dtype), sh)
        for z in zero_outs]

    def run_fn():
        outs = sharded(*dev_in, *dev_zeros)
        import jax as _j
        _j.block_until_ready(outs)
        return outs

    return run_fn, out_names, out_avals


def kernel(**inputs):
    in_maps, T = _host_prep(inputs)
    nc = build_program(T)
    _legalize_waits(nc)
    run_fn, out_names, out_avals = _pjrt_prepare(nc, in_maps)
    outs = run_fn()   # compile + first exec
    if os.environ.get("GAT_BENCH", "0") == "1":
        import time
        times = []
        for _ in range(int(os.environ.get("GAT_BENCH_ITERS", "5"))):
            t0 = time.perf_counter()
            outs = run_fn()
            times.append(time.perf_counter() - t0)
        kernel.last_exec_time_ns = int(min(times) * 1e9)
        kernel.bench_times = times
    i = out_names.index("out")
    full = np.asarray(outs[i]).reshape(NCORES, *out_avals[i].shape)
    return np.asarray(full[0], np.float32)
